# revision 23
# baseline (speedup 1.0000x reference)
"""Trainium2 Bass kernel for a bidirectional cross-attention layer (v6).

Per batch sample (one NeuronCore each, 8 samples / 8 cores):
    e  = seq_1 @ seq_2^T                     [L, L]
    P  = exp(e)            (no max-subtraction: |e| <~ 70 << fp32 overflow)
    seq_1_hat = diag(1/rowsum(P)) @ P   @ seq_2
    seq_2_hat = diag(1/colsum(P)) @ P^T @ seq_1

v6: colsum rides the PE as per-block ones-matvecs accumulating into a
single PSUM bank (one [1,512] accumulator per j-quarter, parked at
partitions 0/32/64/96 of the same bank via 32-aligned tile positions).
That keeps the PE dense (scores + o2 + colsum ~2.6us/block, holding
the 2.4 GHz p-state) while ACT runs the exps and DVE only does the
rowsum accumulation.  o1 runs as a phase B overlapped with both
output epilogues; o2 normalization uses ACT activation-with-scale.
"""

import os

os.environ.setdefault("MYCRO_LOCAL_CACHE", "1")

import numpy as np

import concourse.mybir as mybir
from concourse import bacc
from concourse.bass_utils import run_bass_kernel_spmd
from concourse.tile import TileContext

B, L, D = 8, 2048, 128
NBLK = L // 128  # 16 blocks of 128
NCH = L // 512   # 4 chunks of 512

F32 = mybir.dt.float32
BF16 = mybir.dt.bfloat16
AF = mybir.ActivationFunctionType
ALU = mybir.AluOpType
AX = mybir.AxisListType


def _build():
    nc = bacc.Bacc(
        "TRN2", target_bir_lowering=False, debug=False, enable_asserts=False
    )
    s1 = nc.dram_tensor("seq_1", [L, D], F32, kind="ExternalInput").ap()
    s2 = nc.dram_tensor("seq_2", [L, D], F32, kind="ExternalInput").ap()
    o1 = nc.dram_tensor("out1", [L, D], F32, kind="ExternalOutput").ap()
    o2 = nc.dram_tensor("out2", [L, D], F32, kind="ExternalOutput").ap()

    with TileContext(nc) as tc:
        with (
            tc.tile_pool(name="big", bufs=1) as big,
            tc.tile_pool(name="pbp", bufs=4) as pbp,
            tc.tile_pool(name="scrp", bufs=2) as scrp,
            tc.tile_pool(name="outp", bufs=3) as outp,
            tc.tile_pool(name="etp", bufs=2, space="PSUM") as etp,
            tc.tile_pool(name="acc2p", bufs=1, space="PSUM") as acc2p,
            tc.tile_pool(name="mvp", bufs=1, space="PSUM") as mvp,
            tc.tile_pool(name="acc1p", bufs=1, space="PSUM") as acc1p,
        ):
            # ---- persistent SBUF tensors -------------------------------
            s1f = big.tile([128, L], F32, tag="s1f")    # [i%128, (blk,d)]
            s2f = big.tile([128, L], F32, tag="s2f")
            s1b = big.tile([128, L], BF16, tag="s1b")   # bf16 casts
            s2b = big.tile([128, L], BF16, tag="s2b")
            s1t = big.tile([128, NBLK, 128], BF16, tag="s1t")  # [d, blk, i%128]
            s2t = big.tile([128, NBLK, 128], BF16, tag="s2t")
            ptp = big.tile([128, NBLK, L], BF16, tag="ptp")  # [j%128, jblk, i]
            ones = big.tile([128, 1], BF16, tag="ones")
            rsum = big.tile([128, NBLK], F32, tag="rsum")  # per-block rowsums
            rrow = big.tile([128, NBLK], F32, tag="rrow")
            csumF = big.tile([1, L], F32, tag="csumF")
            rcolraw = big.tile([128, NBLK], F32, tag="rcolraw")
            rcol = big.tile([128, NBLK], F32, tag="rcol")

            nc.gpsimd.memset(ones, 1.0)

            # ---- preload -----------------------------------------------
            # s2 via SP loads + DVE casts (earliest-needed path); s1 via
            # GpSimd SWDGE casting DMAs straight to bf16.  s2t XBARs on
            # ACT (idle pre-exp); s1t XBARs on SP.
            for g in range(4):
                sl = slice(g * 512, (g + 1) * 512)
                nc.sync.dma_start(
                    s2f[:, sl].rearrange("p (blk d) -> p blk d", blk=4),
                    s2[sl, :].rearrange("(blk p) d -> p blk d", blk=4),
                )
            for g in range(4):
                sl = slice(g * 512, (g + 1) * 512)
                nc.gpsimd.dma_start(
                    s1f[:, sl].rearrange("p (blk d) -> p blk d", blk=4),
                    s1[sl, :].rearrange("(blk p) d -> p blk d", blk=4),
                )
            for g in range(4):
                sl = slice(g * 512, (g + 1) * 512)
                nc.vector.tensor_copy(s2b[:, sl], s2f[:, sl])
                nc.scalar.dma_start(
                    s2t[:, 4 * g:4 * g + 4, :], s2b[:, sl], transpose=True
                )
                nc.vector.tensor_copy(s1b[:, sl], s1f[:, sl])
                nc.scalar.dma_start(
                    s1t[:, 4 * g:4 * g + 4, :], s1b[:, sl], transpose=True
                )

            acc2 = acc2p.tile([128, L], F32, tag="acc2")
            # colsum accumulators: two [1,512] per PSUM bank at partitions
            # {0, 64}; the second bank is acc1's (free until phase B, the
            # pool WAR dependency sequences the handoff)
            mvacc = mvp.tile([128, 512], F32, tag="mvacc")
            mvacc2 = acc1p.tile([128, 512], F32, tag="acc1")
            mvq = [(mvacc, 0), (mvacc, 64), (mvacc2, 0), (mvacc2, 64)]

            # ---- fused main phase --------------------------------------
            for b in range(NBLK):
                bsl = slice(b * 128, (b + 1) * 128)
                pb = pbp.tile([128, L], BF16, tag="pb")
                for q in range(4):
                    qsl = slice(q * 512, (q + 1) * 512)
                    et = etp.tile([128, 512], F32, tag="et")
                    nc.tensor.matmul(
                        et, lhsT=s1t[:, b, :], rhs=s2t[:, 4 * q:4 * q + 4, :],
                        start=True, stop=True,
                    )
                    nc.scalar.activation(pb[:, qsl], et, AF.Exp)
                for q in range(4):
                    qsl = slice(q * 512, (q + 1) * 512)
                    nc.tensor.matmul(
                        acc2[:, qsl],
                        lhsT=s1b[:, bsl],
                        rhs=pb[:, qsl],
                        start=(b == 0), stop=(b == NBLK - 1),
                    )
                # colsum partials on PE: ones-matvec per j-quarter into
                # [1,512] accumulators at partitions {0,64} of two banks;
                # partition-0 ones first so the PE tile position flips
                # only once per block
                for q in (0, 2, 1, 3):
                    qsl = slice(q * 512, (q + 1) * 512)
                    mt, mp = mvq[q]
                    nc.tensor.matmul(
                        mt[mp:mp + 1, :],
                        lhsT=ones, rhs=pb[:, qsl],
                        start=(b == 0), stop=(b == NBLK - 1),
                    )
                # rowsum for this block (DVE)
                scr = scrp.tile([128, L], BF16, tag="scr")
                nc.vector.tensor_scalar(
                    scr, pb, 1.0, 0.0, op0=ALU.mult, op1=ALU.add,
                    accum_out=rsum[:, b:b + 1],
                )
                nc.sync.dma_start(ptp[:, :, bsl], pb, transpose=True)

            # ---- phase B: colsum extract, o1 chunks, both epilogues ----
            # colsum: ACT copies each [1,512] quarter into csumF with a
            # permuted AP (element j=c*128+p lands at address p*16+c) so
            # the redistribute DMA below is a plain contiguous-split copy
            csumFv = csumF.rearrange("q (p c) -> q c p", p=128)
            for q in range(4):
                mt, mp = mvq[q]
                nc.scalar.activation(
                    csumFv[:, 4 * q:4 * q + 4, :],
                    mt[mp:mp + 1, :], AF.Copy,
                )
            nc.sync.dma_start(rcolraw, csumF)
            nc.vector.reciprocal(rcol, rcolraw)
            nc.vector.reciprocal(rrow, rsum)

            # o1 chunk GEMMs with both epilogues staggered one chunk
            # behind, so the in-order DVE/ACT queues never block on an
            # XBAR that hasn't finished yet.
            tb1s, tb2s = [], []

            def late_epilogue(k):
                tb1, tb2 = tb1s[k], tb2s[k]
                ksl = slice(k * 512, (k + 1) * 512)
                of1 = outp.tile([128, 512], F32, tag="of1")
                for c2 in range(4):
                    blk = 4 * k + c2
                    nc.vector.tensor_scalar_mul(
                        of1[:, c2 * 128:(c2 + 1) * 128],
                        tb1[:, c2, :], rrow[:, blk:blk + 1],
                    )
                nc.gpsimd.dma_start(
                    o1[ksl, :].rearrange("(c p) d -> p c d", c=4),
                    of1.rearrange("p (c d) -> p c d", c=4),
                )
                of2 = outp.tile([128, 512], F32, tag="of2")
                for c2 in range(4):
                    blk = 4 * k + c2
                    nc.scalar.activation(
                        of2[:, c2 * 128:(c2 + 1) * 128], tb2[:, c2, :],
                        AF.Copy, scale=rcol[:, blk:blk + 1],
                    )
                nc.gpsimd.dma_start(
                    o2[ksl, :].rearrange("(c p) d -> p c d", c=4),
                    of2.rearrange("p (c d) -> p c d", c=4),
                )

            for k in range(NCH):
                # o1 chunk k: o1^T[d, i-chunk] = sum_c s2b[:,c]^T @ ptp
                ksl = slice(k * 512, (k + 1) * 512)
                acc1 = acc1p.tile([128, 512], F32, tag="acc1")
                for c in range(NBLK):
                    nc.tensor.matmul(
                        acc1,
                        lhsT=s2b[:, c * 128:(c + 1) * 128],
                        rhs=ptp[:, c, ksl],
                        start=(c == 0), stop=(c == NBLK - 1),
                    )
                bb1 = outp.tile([128, 512], BF16, tag="bb1")
                nc.vector.tensor_copy(bb1, acc1)
                tb1 = outp.tile([128, 4, 128], BF16, tag="tb1")
                nc.sync.dma_start(tb1, bb1, transpose=True)
                tb1s.append(tb1)
                bb2 = outp.tile([128, 512], BF16, tag="bb2")
                nc.vector.tensor_copy(bb2, acc2[:, ksl])
                tb2 = outp.tile([128, 4, 128], BF16, tag="tb2")
                nc.sync.dma_start(tb2, bb2, transpose=True)
                tb2s.append(tb2)
                if k >= 1:
                    late_epilogue(k - 1)
            late_epilogue(NCH - 1)

    nc.compile()
    return nc


_nc_cache = None


def _run(seq_1, seq_2, trace=False):
    global _nc_cache
    if _nc_cache is None:
        _nc_cache = _build()
    nc = _nc_cache
    seq_1 = np.ascontiguousarray(np.asarray(seq_1, dtype=np.float32))
    seq_2 = np.ascontiguousarray(np.asarray(seq_2, dtype=np.float32))
    in_maps = [{"seq_1": seq_1[b], "seq_2": seq_2[b]} for b in range(B)]
    res = run_bass_kernel_spmd(nc, in_maps, core_ids=list(range(B)), trace=trace)
    out1 = np.stack([res.results[b]["out1"] for b in range(B)])
    out2 = np.stack([res.results[b]["out2"] for b in range(B)])
    return (out1, out2), res


def kernel(seq_1, seq_2):
    return _run(seq_1, seq_2)[0]


# revision 31
# speedup vs baseline: 1.2265x; 1.2265x over previous
"""Trainium2 Bass kernel for a bidirectional cross-attention layer (v8).

Per batch sample (one NeuronCore each, 8 samples / 8 cores):
    e  = seq_1 @ seq_2^T                     [L, L]
    P  = exp(e)            (no max-subtraction: |e| <~ 70 << fp32 overflow)
    seq_1_hat = diag(1/rowsum(P)) @ P   @ seq_2
    seq_2_hat = diag(1/colsum(P)) @ P^T @ seq_1

v8: phase A uses wide [128,1024] exps with the ACT accumulator for
rowsums (cheapest home: +1 accumulator read per activation) and DVE
tensor_reduce over the transposed P stripes for colsum partials; PE
runs only the score + o2 GEMMs, staying ~90% dense.  The preload
pipelines loads/casts/XBARs at 512-row granularity across the SP/ACT
queues so the first scores issue ~15us in.  Phase B runs the o1 chunk
GEMMs (LDWEIGHTS fully hidden in the single-bank accumulation runs)
with both outputs' transpose/normalize/store epilogues staggered one
chunk behind across DVE/ACT/SP/Pool so nothing blocks in-order queues.
"""

import os

os.environ.setdefault("MYCRO_LOCAL_CACHE", "1")

import numpy as np

import concourse.mybir as mybir
from concourse import bacc
from concourse.bass_utils import run_bass_kernel_spmd
from concourse.tile import TileContext

B, L, D = 8, 2048, 128
NBLK = L // 128  # 16 blocks of 128
NCH = L // 512   # 4 chunks of 512

F32 = mybir.dt.float32
BF16 = mybir.dt.bfloat16
AF = mybir.ActivationFunctionType
ALU = mybir.AluOpType
AX = mybir.AxisListType


def _build():
    nc = bacc.Bacc(
        "TRN2", target_bir_lowering=False, debug=False, enable_asserts=False
    )
    s1 = nc.dram_tensor("seq_1", [L, D], F32, kind="ExternalInput").ap()
    s2 = nc.dram_tensor("seq_2", [L, D], F32, kind="ExternalInput").ap()
    o1 = nc.dram_tensor("out1", [L, D], F32, kind="ExternalOutput").ap()
    o2 = nc.dram_tensor("out2", [L, D], F32, kind="ExternalOutput").ap()

    with TileContext(nc) as tc:
        with (
            tc.tile_pool(name="big", bufs=1) as big,
            tc.tile_pool(name="pbp", bufs=4) as pbp,
            tc.tile_pool(name="outp", bufs=3) as outp,
            tc.tile_pool(name="acc2p", bufs=1, space="PSUM") as acc2p,
        ):
            # ---- persistent SBUF tensors -------------------------------
            s1f = big.tile([128, L], F32, tag="s1f")    # [i%128, (blk,d)]
            s2f = big.tile([128, L], F32, tag="s2f")
            s1b = big.tile([128, L], BF16, tag="s1b")   # bf16 casts
            s2b = big.tile([128, L], BF16, tag="s2b")
            s1t = big.tile([128, NBLK, 128], BF16, tag="s1t")  # [d, blk, i%128]
            s2t = big.tile([128, NBLK, 128], BF16, tag="s2t")
            ptp = big.tile([128, NBLK, L], BF16, tag="ptp")  # [j%128, jblk, i]
            rsum4 = big.tile([128, NBLK * 2], F32, tag="rsum4")
            csum4 = big.tile([128, NBLK * NBLK], F32, tag="csum4")
            rsum = big.tile([128, NBLK], F32, tag="rsum")
            colsum = big.tile([128, NBLK], F32, tag="colsum")
            rrow = big.tile([128, NBLK], F32, tag="rrow")
            rcol = big.tile([128, NBLK], F32, tag="rcol")

            # ---- preload (v3's serial single-queue form: race-free) ----
            for t_dram, t_f in ((s2, s2f), (s1, s1f)):
                for g in range(4):
                    sl = slice(g * 512, (g + 1) * 512)
                    nc.sync.dma_start(
                        t_f[:, sl].rearrange("p (blk d) -> p blk d", blk=4),
                        t_dram[sl, :].rearrange("(blk p) d -> p blk d", blk=4),
                    )
            for t_f, t_b, t_t in ((s2f, s2b, s2t), (s1f, s1b, s1t)):
                for g in range(4):
                    sl = slice(g * 512, (g + 1) * 512)
                    nc.vector.tensor_copy(t_b[:, sl], t_f[:, sl])
                nc.sync.dma_start(t_t[:, 0:8, :], t_b[:, :1024], transpose=True)
                nc.sync.dma_start(t_t[:, 8:16, :], t_b[:, 1024:], transpose=True)

            acc2 = acc2p.tile([128, L], F32, tag="acc2")

            # ---- phase A: scores, exp(+rowsum), o2 accum, P^T, colsum --
            with tc.tile_pool(name="ep", bufs=2, space="PSUM") as ep:
                for b in range(NBLK):
                    bsl = slice(b * 128, (b + 1) * 128)
                    pb = pbp.tile([128, L], BF16, tag="pb")
                    for h in range(2):
                        et = ep.tile([128, 1024], F32, tag="et")
                        for q in range(2):
                            a = 2 * h + q
                            nc.tensor.matmul(
                                et[:, q * 512:(q + 1) * 512],
                                lhsT=s1t[:, b, :],
                                rhs=s2t[:, 4 * a:4 * a + 4, :],
                                start=True, stop=True,
                            )
                        nc.scalar.activation(
                            pb[:, h * 1024:(h + 1) * 1024], et, AF.Exp,
                            accum_out=rsum4[:, 2 * b + h:2 * b + h + 1],
                        )
                    for q in range(4):
                        qsl = slice(q * 512, (q + 1) * 512)
                        nc.tensor.matmul(
                            acc2[:, qsl],
                            lhsT=s1b[:, bsl],
                            rhs=pb[:, qsl],
                            start=(b == 0), stop=(b == NBLK - 1),
                        )
                    nc.sync.dma_start(ptp[:, :, bsl], pb, transpose=True)
                    # colsum partial reads the PREVIOUS block's P^T stripe:
                    # one block of slack between the XBAR write and the DVE
                    # read (reading a just-completed XBAR stripe proved
                    # racy on hardware)
                    if b >= 1:
                        psl = slice((b - 1) * 128, b * 128)
                        nc.vector.tensor_reduce(
                            csum4[:, (b - 1) * NBLK:b * NBLK],
                            ptp[:, :, psl], axis=AX.X, op=ALU.add,
                        )

                # last colsum partial, then folds and reciprocals
                lsl = slice((NBLK - 1) * 128, NBLK * 128)
                nc.vector.tensor_reduce(
                    csum4[:, (NBLK - 1) * NBLK:NBLK * NBLK],
                    ptp[:, :, lsl], axis=AX.X, op=ALU.add,
                )
                nc.vector.tensor_reduce(
                    rsum, rsum4.rearrange("p (b t) -> p b t", b=NBLK),
                    axis=AX.X, op=ALU.add,
                )
                nc.vector.reciprocal(rrow, rsum)
                nc.vector.tensor_reduce(
                    colsum, csum4.rearrange("p (b c) -> p c b", b=NBLK),
                    axis=AX.X, op=ALU.add,
                )
                nc.vector.reciprocal(rcol, colsum)

            # ---- phase B: o1 chunks + staggered epilogues --------------
            # Chunk 3 first: its matmuls depend on the LAST XBAR stripe,
            # which naturally drains the et pipeline before acc1 reuses
            # those PSUM banks (a ready-instantly chunk first races that
            # bank reuse on hardware).
            tb1s, tb2s = {}, {}

            def late_epilogue(k):
                tb1, tb2 = tb1s[k], tb2s[k]
                ksl = slice(k * 512, (k + 1) * 512)
                of1 = outp.tile([128, 512], F32, tag="of1")
                for c2 in range(4):
                    blk = 4 * k + c2
                    nc.vector.tensor_scalar_mul(
                        of1[:, c2 * 128:(c2 + 1) * 128],
                        tb1[:, c2, :], rrow[:, blk:blk + 1],
                    )
                nc.gpsimd.dma_start(
                    o1[ksl, :].rearrange("(c p) d -> p c d", c=4),
                    of1.rearrange("p (c d) -> p c d", c=4),
                )
                of2 = outp.tile([128, 512], F32, tag="of2")
                for c2 in range(4):
                    blk = 4 * k + c2
                    nc.scalar.activation(
                        of2[:, c2 * 128:(c2 + 1) * 128], tb2[:, c2, :],
                        AF.Copy, scale=rcol[:, blk:blk + 1],
                    )
                nc.gpsimd.dma_start(
                    o2[ksl, :].rearrange("(c p) d -> p c d", c=4),
                    of2.rearrange("p (c d) -> p c d", c=4),
                )

            korder = (3, 0, 1, 2)
            with tc.tile_pool(name="acc1p", bufs=2, space="PSUM") as acc1p:
                for j, k in enumerate(korder):
                    ksl = slice(k * 512, (k + 1) * 512)
                    acc1 = acc1p.tile([128, 512], F32, tag="acc1")
                    for c in range(NBLK):
                        nc.tensor.matmul(
                            acc1,
                            lhsT=s2b[:, c * 128:(c + 1) * 128],
                            rhs=ptp[:, c, ksl],
                            start=(c == 0), stop=(c == NBLK - 1),
                        )
                    bb1 = outp.tile([128, 512], BF16, tag="bb1")
                    nc.vector.tensor_copy(bb1, acc1)
                    tb1 = outp.tile([128, 4, 128], BF16, tag="tb1")
                    nc.sync.dma_start(tb1, bb1, transpose=True)
                    tb1s[k] = tb1
                    bb2 = outp.tile([128, 512], BF16, tag="bb2")
                    nc.vector.tensor_copy(bb2, acc2[:, ksl])
                    tb2 = outp.tile([128, 4, 128], BF16, tag="tb2")
                    nc.sync.dma_start(tb2, bb2, transpose=True)
                    tb2s[k] = tb2
                    if j >= 1:
                        late_epilogue(korder[j - 1])
                late_epilogue(korder[-1])

    nc.compile()
    return nc


_nc_cache = None


def _run(seq_1, seq_2, trace=False):
    global _nc_cache
    if _nc_cache is None:
        _nc_cache = _build()
    nc = _nc_cache
    seq_1 = np.ascontiguousarray(np.asarray(seq_1, dtype=np.float32))
    seq_2 = np.ascontiguousarray(np.asarray(seq_2, dtype=np.float32))
    in_maps = [{"seq_1": seq_1[b], "seq_2": seq_2[b]} for b in range(B)]
    res = run_bass_kernel_spmd(nc, in_maps, core_ids=list(range(B)), trace=trace)
    out1 = np.stack([res.results[b]["out1"] for b in range(B)])
    out2 = np.stack([res.results[b]["out2"] for b in range(B)])
    return (out1, out2), res


def kernel(seq_1, seq_2):
    return _run(seq_1, seq_2)[0]


# revision 33
# speedup vs baseline: 1.3273x; 1.0821x over previous
"""Trainium2 Bass kernel for a bidirectional cross-attention layer (v8).

Per batch sample (one NeuronCore each, 8 samples / 8 cores):
    e  = seq_1 @ seq_2^T                     [L, L]
    P  = exp(e)            (no max-subtraction: |e| <~ 70 << fp32 overflow)
    seq_1_hat = diag(1/rowsum(P)) @ P   @ seq_2
    seq_2_hat = diag(1/colsum(P)) @ P^T @ seq_1

v8: phase A uses wide [128,1024] exps with the ACT accumulator for
rowsums (cheapest home: +1 accumulator read per activation) and DVE
tensor_reduce over the transposed P stripes for colsum partials; PE
runs only the score + o2 GEMMs, staying ~90% dense.  The preload
pipelines loads/casts/XBARs at 512-row granularity across the SP/ACT
queues so the first scores issue ~15us in.  Phase B runs the o1 chunk
GEMMs (LDWEIGHTS fully hidden in the single-bank accumulation runs)
with both outputs' transpose/normalize/store epilogues staggered one
chunk behind across DVE/ACT/SP/Pool so nothing blocks in-order queues.
"""

import os

os.environ.setdefault("MYCRO_LOCAL_CACHE", "1")

import numpy as np

import concourse.mybir as mybir
from concourse import bacc
from concourse.bass_utils import run_bass_kernel_spmd
from concourse.tile import TileContext

B, L, D = 8, 2048, 128
NBLK = L // 128  # 16 blocks of 128
NCH = L // 512   # 4 chunks of 512

F32 = mybir.dt.float32
BF16 = mybir.dt.bfloat16
AF = mybir.ActivationFunctionType
ALU = mybir.AluOpType
AX = mybir.AxisListType


def _build():
    nc = bacc.Bacc(
        "TRN2", target_bir_lowering=False, debug=False, enable_asserts=False
    )
    s1 = nc.dram_tensor("seq_1", [L, D], F32, kind="ExternalInput").ap()
    s2 = nc.dram_tensor("seq_2", [L, D], F32, kind="ExternalInput").ap()
    o1 = nc.dram_tensor("out1", [L, D], F32, kind="ExternalOutput").ap()
    o2 = nc.dram_tensor("out2", [L, D], F32, kind="ExternalOutput").ap()

    with TileContext(nc) as tc:
        with (
            tc.tile_pool(name="big", bufs=1) as big,
            tc.tile_pool(name="pbp", bufs=4) as pbp,
            tc.tile_pool(name="outp", bufs=3) as outp,
            tc.tile_pool(name="acc2p", bufs=1, space="PSUM") as acc2p,
        ):
            # ---- persistent SBUF tensors -------------------------------
            s1f = big.tile([128, L], F32, tag="s1f")    # [i%128, (blk,d)]
            s2f = big.tile([128, L], F32, tag="s2f")
            s1b = big.tile([128, L], BF16, tag="s1b")   # bf16 casts
            s2b = big.tile([128, L], BF16, tag="s2b")
            s1t = big.tile([128, NBLK, 128], BF16, tag="s1t")  # [d, blk, i%128]
            s2t = big.tile([128, NBLK, 128], BF16, tag="s2t")
            ptp = big.tile([128, NBLK, L], BF16, tag="ptp")  # [j%128, jblk, i]
            rsum4 = big.tile([128, NBLK * 2], F32, tag="rsum4")
            csum4 = big.tile([128, NBLK * NBLK], F32, tag="csum4")
            rsum = big.tile([128, NBLK], F32, tag="rsum")
            colsum = big.tile([128, NBLK], F32, tag="colsum")
            rrow = big.tile([128, NBLK], F32, tag="rrow")
            rcol = big.tile([128, NBLK], F32, tag="rcol")

            # ---- preload (serial single-queue form: race-free) ---------
            # All loads and XBARs stay on the SP queue in dependency
            # order; the first-half XBARs of both tensors go before the
            # second halves so block 0's scores can start sooner.
            for t_dram, t_f in ((s2, s2f), (s1, s1f)):
                for g in range(4):
                    sl = slice(g * 512, (g + 1) * 512)
                    nc.sync.dma_start(
                        t_f[:, sl].rearrange("p (blk d) -> p blk d", blk=4),
                        t_dram[sl, :].rearrange("(blk p) d -> p blk d", blk=4),
                    )
            for t_f, t_b in ((s2f, s2b), (s1f, s1b)):
                for g in range(2):
                    sl = slice(g * 512, (g + 1) * 512)
                    nc.vector.tensor_copy(t_b[:, sl], t_f[:, sl])
            for t_f, t_b in ((s2f, s2b), (s1f, s1b)):
                for g in range(2, 4):
                    sl = slice(g * 512, (g + 1) * 512)
                    nc.vector.tensor_copy(t_b[:, sl], t_f[:, sl])
            nc.sync.dma_start(s2t[:, 0:8, :], s2b[:, :1024], transpose=True)
            nc.sync.dma_start(s1t[:, 0:8, :], s1b[:, :1024], transpose=True)
            nc.sync.dma_start(s2t[:, 8:16, :], s2b[:, 1024:], transpose=True)
            nc.sync.dma_start(s1t[:, 8:16, :], s1b[:, 1024:], transpose=True)

            acc2 = acc2p.tile([128, L], F32, tag="acc2")

            # ---- phase A: scores, exp(+rowsum), o2 accum, P^T, colsum --
            with tc.tile_pool(name="ep", bufs=2, space="PSUM") as ep:
                for b in range(NBLK):
                    bsl = slice(b * 128, (b + 1) * 128)
                    pb = pbp.tile([128, L], BF16, tag="pb")
                    for h in range(2):
                        et = ep.tile([128, 1024], F32, tag="et")
                        for q in range(2):
                            a = 2 * h + q
                            nc.tensor.matmul(
                                et[:, q * 512:(q + 1) * 512],
                                lhsT=s1t[:, b, :],
                                rhs=s2t[:, 4 * a:4 * a + 4, :],
                                start=True, stop=True,
                            )
                        nc.scalar.activation(
                            pb[:, h * 1024:(h + 1) * 1024], et, AF.Exp,
                            accum_out=rsum4[:, 2 * b + h:2 * b + h + 1],
                        )
                    for q in range(4):
                        qsl = slice(q * 512, (q + 1) * 512)
                        nc.tensor.matmul(
                            acc2[:, qsl],
                            lhsT=s1b[:, bsl],
                            rhs=pb[:, qsl],
                            start=(b == 0), stop=(b == NBLK - 1),
                        )
                    nc.sync.dma_start(ptp[:, :, bsl], pb, transpose=True)
                    # colsum partial reads the PREVIOUS block's P^T stripe:
                    # one block of slack between the XBAR write and the DVE
                    # read (reading a just-completed XBAR stripe proved
                    # racy on hardware)
                    if b >= 1:
                        psl = slice((b - 1) * 128, b * 128)
                        nc.vector.tensor_reduce(
                            csum4[:, (b - 1) * NBLK:b * NBLK],
                            ptp[:, :, psl], axis=AX.X, op=ALU.add,
                        )

                # last colsum partial, then folds and reciprocals
                lsl = slice((NBLK - 1) * 128, NBLK * 128)
                nc.vector.tensor_reduce(
                    csum4[:, (NBLK - 1) * NBLK:NBLK * NBLK],
                    ptp[:, :, lsl], axis=AX.X, op=ALU.add,
                )
                nc.vector.tensor_reduce(
                    rsum, rsum4.rearrange("p (b t) -> p b t", b=NBLK),
                    axis=AX.X, op=ALU.add,
                )
                nc.vector.reciprocal(rrow, rsum)
                nc.vector.tensor_reduce(
                    colsum, csum4.rearrange("p (b c) -> p c b", b=NBLK),
                    axis=AX.X, op=ALU.add,
                )
                nc.vector.reciprocal(rcol, colsum)

            # ---- phase B ------------------------------------------------
            # o2's epilogue is front-loaded (acc2 is complete): bb2 copies
            # + XBARs + ACT scale-normalize + stores all run while the PE
            # does the o1 chunk GEMMs.  Chunk 3 first: its matmuls depend
            # on the LAST XBAR stripe, which naturally drains the et
            # pipeline before acc1 reuses those PSUM banks.
            korder = (3, 0, 1, 2)
            with tc.tile_pool(name="acc1p", bufs=2, space="PSUM") as acc1p:
                # o2 epilogue, fully parallel to the o1 GEMMs below
                for k in range(NCH):
                    ksl = slice(k * 512, (k + 1) * 512)
                    bb2 = outp.tile([128, 512], BF16, tag="bb2")
                    nc.vector.tensor_copy(bb2, acc2[:, ksl])
                    tb2 = outp.tile([128, 4, 128], BF16, tag="tb2")
                    nc.sync.dma_start(tb2, bb2, transpose=True)
                    of2 = outp.tile([128, 512], F32, tag="of2")
                    for c2 in range(4):
                        blk = 4 * k + c2
                        nc.scalar.activation(
                            of2[:, c2 * 128:(c2 + 1) * 128], tb2[:, c2, :],
                            AF.Copy, scale=rcol[:, blk:blk + 1],
                        )
                    nc.sync.dma_start(
                        o2[ksl, :].rearrange("(c p) d -> p c d", c=4),
                        of2.rearrange("p (c d) -> p c d", c=4),
                    )

                tb1s = {}

                def o1_epilogue(k):
                    tb1 = tb1s[k]
                    ksl = slice(k * 512, (k + 1) * 512)
                    of1 = outp.tile([128, 512], F32, tag="of1")
                    for c2 in range(4):
                        blk = 4 * k + c2
                        nc.vector.tensor_scalar_mul(
                            of1[:, c2 * 128:(c2 + 1) * 128],
                            tb1[:, c2, :], rrow[:, blk:blk + 1],
                        )
                    nc.sync.dma_start(
                        o1[ksl, :].rearrange("(c p) d -> p c d", c=4),
                        of1.rearrange("p (c d) -> p c d", c=4),
                    )

                for j, k in enumerate(korder):
                    ksl = slice(k * 512, (k + 1) * 512)
                    acc1 = acc1p.tile([128, 512], F32, tag="acc1")
                    for c in range(NBLK):
                        nc.tensor.matmul(
                            acc1,
                            lhsT=s2b[:, c * 128:(c + 1) * 128],
                            rhs=ptp[:, c, ksl],
                            start=(c == 0), stop=(c == NBLK - 1),
                        )
                    bb1 = outp.tile([128, 512], BF16, tag="bb1")
                    nc.vector.tensor_copy(bb1, acc1)
                    tb1 = outp.tile([128, 4, 128], BF16, tag="tb1")
                    nc.sync.dma_start(tb1, bb1, transpose=True)
                    tb1s[k] = tb1
                    if j >= 1:
                        o1_epilogue(korder[j - 1])
                o1_epilogue(korder[-1])

    nc.compile()
    return nc


_nc_cache = None


def _run(seq_1, seq_2, trace=False):
    global _nc_cache
    if _nc_cache is None:
        _nc_cache = _build()
    nc = _nc_cache
    seq_1 = np.ascontiguousarray(np.asarray(seq_1, dtype=np.float32))
    seq_2 = np.ascontiguousarray(np.asarray(seq_2, dtype=np.float32))
    in_maps = [{"seq_1": seq_1[b], "seq_2": seq_2[b]} for b in range(B)]
    res = run_bass_kernel_spmd(nc, in_maps, core_ids=list(range(B)), trace=trace)
    out1 = np.stack([res.results[b]["out1"] for b in range(B)])
    out2 = np.stack([res.results[b]["out2"] for b in range(B)])
    return (out1, out2), res


def kernel(seq_1, seq_2):
    return _run(seq_1, seq_2)[0]
